# revision 7
# baseline (speedup 1.0000x reference)
"""GQA causal attention on 8 Trainium2 NeuronCores (Bass/Tile).

Problem: x[4,2048,2048] -> QKV proj (NH=16 q-heads, NKV=4 kv-heads, HD=128)
-> causal softmax attention -> out proj.

Sharding: core c handles batch b=c//2 and head-half h=c%2 (8 q-heads,
2 kv-heads).  Each core computes a partial output-projection
yT_part = (out_half @ Wo[rows-half]).T; the host sums the two partials per
batch (+ bo).  No on-device collectives.

Per-core kernel (all matmuls fp32r at moving-N>=256 => 1 cycle/row):
  phase 1: QT[f,s], KT[f,s], VT->V[s,hd] projections from xT tiles
           (contraction d on partitions; two 6-psum-bank sweeps with the
           sweep's weight block resident in SBUF)
  phase 2: per (head, q-chunk of 512):
             scoresT[k,q] = KT_tile^T @ QT_chunk  (128x512 psum tiles)
             probsT = exp(scoresT)   (no max-sub needed: |s| <~ 6)
             causal: multiply diagonal tiles by precomputed 0/1 masks
             l[1,q]   += ones^T @ probsT     (PE partition-reduction)
             av[hd,q] += V_tile^T @ probsT
             outT[:,h,q] = av * gpsimd_bcast(1/l)  (off the PE queue)
  phase 3: yT[n,s] = sum_f Wo_tile[f,n]^T @ outT[:,f,s]  (Wo streamed
           per n-tile, prefetched during phase 2)
1/sqrt(HD) is folded into Wq/bq on the host.
"""

import math
import sys
from contextlib import ExitStack

import numpy as np

if "/opt/trn_rl_repo" not in sys.path:
    sys.path.insert(0, "/opt/trn_rl_repo")

B, S, D = 4, 2048, 2048
NH, NKV, HD = 16, 4, 128
FH, KVH = 8, 2  # per-core q heads / kv heads
FW = FH * HD    # 1024, per-core q feature width
KW = KVH * HD   # 256, per-core kv feature width
NCORE = 8
SCALE = 1.0 / math.sqrt(HD)

NDT = D // 128   # 16 contraction tiles
NSC = S // 512   # 4 s-chunks (phase 1 moving dim)
NKT = S // 128   # 16 k-tiles
NQC = S // 512   # 4 q-chunks
NNT = D // 128   # 16 n-tiles (phase 3)

_CACHE = {}


def build_nc():
    import concourse.mybir as mybir
    import concourse.tile as tile
    from concourse import bacc
    from concourse.masks import make_identity

    f32 = mybir.dt.float32
    f32r = mybir.dt.float32r
    mm = f32r  # matmul operand dtype (flip to mybir.dt.bfloat16 as fallback)
    Exp = mybir.ActivationFunctionType.Exp
    Ident = mybir.ActivationFunctionType.Identity

    nc = bacc.Bacc("TRN2", target_bir_lowering=False, debug=False)

    xT = nc.declare_dram_parameter("xT", [D, S], mm, isOutput=False)
    # pre-tiled weight blocks: [p, d_tile, cols]
    # wa: Wq f-tiles 0..5 ; wb: Wq f-tiles 6,7 | Wk | Wv
    wa = nc.declare_dram_parameter("wa", [128, NDT, 768], mm, isOutput=False)
    wb = nc.declare_dram_parameter("wb", [128, NDT, 768], mm, isOutput=False)
    # wo[p, nt, f, j] = Wo_half[f*128+p, nt*128+j]
    wo = nc.declare_dram_parameter("wo", [HD, NNT, FH, 128], mm, isOutput=False)
    bqp = nc.declare_dram_parameter("bq", [HD, FH], f32, isOutput=False)
    bkp = nc.declare_dram_parameter("bk", [HD, KVH], f32, isOutput=False)
    bvp = nc.declare_dram_parameter("bv", [HD, KVH], f32, isOutput=False)
    maskp = nc.declare_dram_parameter("masks", [HD, 4, 512], f32, isOutput=False)
    onesp = nc.declare_dram_parameter("ones", [HD, 128], mm, isOutput=False)
    # transposed partial output
    y = nc.declare_dram_parameter("y", [D, S], f32, isOutput=True)


    with tile.TileContext(nc) as tc, ExitStack() as ctx:
        persist = ctx.enter_context(tc.tile_pool(name="persist", bufs=1))
        wo_pool = ctx.enter_context(tc.tile_pool(name="wo", bufs=2))
        # one 64KB/partition slot time-shared: sweepA weights -> sweepB weights -> outT
        share = ctx.enter_context(tc.tile_pool(name="share", bufs=1))

        qt_sb = persist.tile([128, FH, S], mm, tag="qt", name="qt_sb")
        kt_sb = persist.tile([128, KVH, S], mm, tag="kt", name="kt_sb")
        v_sb = persist.tile([128, KVH, NKT, HD], mm, tag="v", name="v_sb")
        mask_sb = persist.tile([128, 4, 512], f32, tag="mask", name="mask_sb")
        bq_sb = persist.tile([128, FH], f32, tag="bq", name="bq_sb")
        bk_sb = persist.tile([128, KVH], f32, tag="bk", name="bk_sb")
        bv_sb = persist.tile([128, KVH], f32, tag="bv", name="bv_sb")
        ones_sb = persist.tile([128, 128], mm, tag="ones", name="ones_sb")
        ident_sb = persist.tile([128, 128], f32, tag="ident", name="ident_sb")

        nc.sync.dma_start(mask_sb[:], maskp[:])
        nc.sync.dma_start(bq_sb[:], bqp[:])
        nc.sync.dma_start(bk_sb[:], bkp[:])
        nc.sync.dma_start(bv_sb[:], bvp[:])
        nc.sync.dma_start(ones_sb[:], onesp[:])
        make_identity(nc, ident_sb[:])

        # ---------------- phase 1: projections ----------------
        # sweep 0: q f-tiles 0..5; sweep 1: q 6,7 + k 0,1 + v 0,1
        for sweep_i, wblk in ((0, wa), (1, wb)):
            with (
                tc.tile_pool(name=f"p1ps{sweep_i}", bufs=6, space="PSUM") as proj_pool,
                tc.tile_pool(name=f"p1vt{sweep_i}", bufs=2, space="PSUM") as vt_pool,
                tc.tile_pool(name=f"p1xs{sweep_i}", bufs=6) as xs_pool,
                tc.tile_pool(name=f"p1vtmp{sweep_i}", bufs=2) as vtmp_pool,
            ):
                wsb = share.tile([128, NDT, 768], mm, tag="share", name=f"wsb{sweep_i}")
                for sc in range(NSC):
                    ss = slice(sc * 512, (sc + 1) * 512)
                    ps = [
                        proj_pool.tile([128, 512], f32, tag="proj", name=f"proj{j}")
                        for j in range(6)
                    ]
                    for d in range(NDT):
                        ds = slice(d * 128, (d + 1) * 128)
                        if sc == 0:
                            # just-in-time weight slice so the first matmuls
                            # don't wait behind the whole 6MB block
                            nc.sync.dma_start(wsb[:, d, :], wblk[:, d, :])
                        xs = xs_pool.tile([128, 512], mm, tag="xs", name="xs")
                        nc.sync.dma_start(xs[:], xT[ds, ss])
                        for j in range(6):
                            nc.tensor.matmul(
                                ps[j][:],
                                wsb[:, d, j * 128 : (j + 1) * 128],
                                xs[:],
                                start=(d == 0),
                                stop=(d == NDT - 1),
                            )
                    if sweep_i == 0:
                        for j in range(6):
                            nc.scalar.activation(
                                qt_sb[:, j, ss], ps[j][:], Ident, bias=bq_sb[:, j : j + 1]
                            )
                    else:
                        for j in range(2):
                            nc.scalar.activation(
                                qt_sb[:, 6 + j, ss], ps[j][:], Ident,
                                bias=bq_sb[:, 6 + j : 7 + j],
                            )
                        for kvi in range(KVH):
                            nc.scalar.activation(
                                kt_sb[:, kvi, ss], ps[2 + kvi][:], Ident,
                                bias=bk_sb[:, kvi : kvi + 1],
                            )
                        for kvi in range(KVH):
                            vtmp = vtmp_pool.tile([128, 512], f32, tag="vtmp", name="vtmp")
                            nc.scalar.activation(
                                vtmp[:], ps[4 + kvi][:], Ident,
                                bias=bv_sb[:, kvi : kvi + 1],
                            )
                            for i in range(4):
                                vps = vt_pool.tile([128, 128], f32, tag="vps", name="vps")
                                nc.tensor.transpose(
                                    vps[:], vtmp[:, i * 128 : (i + 1) * 128], ident_sb[:]
                                )
                                nc.vector.tensor_copy(v_sb[:, kvi, sc * 4 + i, :], vps[:])

        # ---------------- phase 2: attention ----------------
        outT_sb = share.tile([128, FH, S], mm, tag="share", name="outT_sb")
        with (
            tc.tile_pool(name="p2sc", bufs=3, space="PSUM") as sc_pool,
            tc.tile_pool(name="p2l", bufs=2, space="PSUM") as l_pool,
            tc.tile_pool(name="p2av", bufs=3, space="PSUM") as av_pool,
            tc.tile_pool(name="p2pt", bufs=3) as pt_pool,
            tc.tile_pool(name="p2lsb", bufs=2) as lsb_pool,
            tc.tile_pool(name="p2bc", bufs=2) as bc_pool,
        ):
            for h in range(FH):
                kv = h // 4
                for qc in range(NQC):
                    qs = slice(qc * 512, (qc + 1) * 512)
                    ktmax = 4 * qc + 3
                    l_ps = l_pool.tile([1, 512], f32, tag="l", name="l_ps")
                    av_ps = av_pool.tile([128, 512], f32, tag="av", name="av_ps")
                    for kt in range(ktmax + 1):
                        sc_ps = sc_pool.tile([128, 512], f32, tag="sc", name="sc_ps")
                        nc.tensor.matmul(
                            sc_ps[:],
                            kt_sb[:, kv, kt * 128 : (kt + 1) * 128],
                            qt_sb[:, h, qs],
                            start=True,
                            stop=True,
                        )
                        j = kt - 4 * qc
                        if j >= 0:
                            nc.vector.tensor_add(sc_ps[:], sc_ps[:], mask_sb[:, j, :])
                        pt = pt_pool.tile([128, 512], mm, tag="pt", name="pt")
                        nc.scalar.activation(pt[:], sc_ps[:], Exp)
                        nc.tensor.matmul(
                            l_ps[:], ones_sb[:, 0:1], pt[:],
                            start=(kt == 0), stop=(kt == ktmax),
                        )
                        nc.tensor.matmul(
                            av_ps[:], v_sb[:, kv, kt, :], pt[:],
                            start=(kt == 0), stop=(kt == ktmax),
                        )
                    rec = lsb_pool.tile([1, 512], f32, tag="rec", name="rec")
                    nc.vector.reciprocal(rec[:], l_ps[:])
                    bc_sb = bc_pool.tile([128, 512], f32, tag="bc", name="bc_sb")
                    nc.gpsimd.partition_broadcast(bc_sb[:], rec[:])
                    nc.vector.tensor_mul(outT_sb[:, h, qs], av_ps[:], bc_sb[:])

        # ---------------- phase 3: output projection (yT) ----------------
        with (
            tc.tile_pool(name="p3ps", bufs=4, space="PSUM") as y_pool,
            tc.tile_pool(name="p3st", bufs=3) as yst_pool,
        ):
            for nt in range(NNT):
                wot = wo_pool.tile([128, FH, 128], mm, tag="wo", name="wot")
                nc.sync.dma_start(wot[:], wo[:, nt, :, :])
                nts = slice(nt * 128, (nt + 1) * 128)
                for sch in range(NSC):
                    ss = slice(sch * 512, (sch + 1) * 512)
                    yps = y_pool.tile([128, 512], f32, tag="yps", name="yps")
                    for f in range(FH):
                        nc.tensor.matmul(
                            yps[:],
                            wot[:, f, :],
                            outT_sb[:, f, ss],
                            start=(f == 0),
                            stop=(f == FH - 1),
                        )
                    ysb = yst_pool.tile([128, 512], f32, tag="ysb", name="ysb")
                    nc.vector.tensor_copy(ysb[:], yps[:])
                    nc.sync.dma_start(y[nts, ss], ysb[:])

    nc.compile()
    return nc


def _masks_np():
    # mask[p, j, q] = 1.0 iff (global k = kt*128+p) <= (global q = qc*512+q)
    # for diagonal tiles where delta = 512*qc - 128*kt = -128*j
    p = np.arange(128)[:, None]
    q = np.arange(512)[None, :]
    m = np.stack([(p <= q - 128 * j) for j in range(4)], axis=1)
    return np.ascontiguousarray(np.where(m, 0.0, -1.0e4).astype(np.float32))


def make_in_maps(x, Wq, bq, Wk, bk, Wv, bv, Wo):
    x = np.asarray(x, np.float32)
    Wq = np.asarray(Wq, np.float32)
    bq = np.asarray(bq, np.float32)
    Wk = np.asarray(Wk, np.float32)
    bk = np.asarray(bk, np.float32)
    Wv = np.asarray(Wv, np.float32)
    bv = np.asarray(bv, np.float32)
    Wo = np.asarray(Wo, np.float32)

    masks = _masks_np()
    xTs = [np.ascontiguousarray(x[b].T) for b in range(B)]
    in_maps = []
    for c in range(NCORE):
        b, h = c // 2, c % 2
        fq = slice(h * FW, (h + 1) * FW)
        fk = slice(h * KW, (h + 1) * KW)
        wq_c = Wq[:, fq] * SCALE  # [D, 1024]
        # [D, cols] -> [128, NDT, cols]
        wa = wq_c[:, 0:768].reshape(NDT, 128, 768).transpose(1, 0, 2)
        wb = np.concatenate([wq_c[:, 768:1024], Wk[:, fk], Wv[:, fk]], axis=1)
        wb = wb.reshape(NDT, 128, 768).transpose(1, 0, 2)
        # Wo rows half -> [p, nt, f, j]
        wo_c = (
            Wo[h * FW : (h + 1) * FW, :]
            .reshape(FH, HD, NNT, 128)
            .transpose(1, 2, 0, 3)
        )
        in_maps.append(
            {
                "xT": xTs[b],
                "wa": np.ascontiguousarray(wa),
                "wb": np.ascontiguousarray(wb),
                "wo": np.ascontiguousarray(wo_c),
                "bq": np.ascontiguousarray((bq[fq] * SCALE).reshape(FH, HD).T),
                "bk": np.ascontiguousarray(bk[fk].reshape(KVH, HD).T),
                "bv": np.ascontiguousarray(bv[fk].reshape(KVH, HD).T),
                "masks": masks,
                "ones": np.ones((HD, 128), np.float32),
            }
        )
    return in_maps


LAST_RESULT = None


def kernel(x, Wq, bq, Wk, bk, Wv, bv, Wo, bo):
    global LAST_RESULT
    from concourse.bass_utils import run_bass_kernel_spmd

    if "nc" not in _CACHE:
        _CACHE["nc"] = build_nc()
    nc = _CACHE["nc"]

    in_maps = make_in_maps(x, Wq, bq, Wk, bk, Wv, bv, Wo)
    res = run_bass_kernel_spmd(nc, in_maps, list(range(NCORE)))
    LAST_RESULT = res

    bo = np.asarray(bo, np.float32)
    out = np.empty((B, S, D), np.float32)
    for b in range(B):
        out[b] = res.results[2 * b]["y"].T + res.results[2 * b + 1]["y"].T + bo[None, :]
    return out
